# revision 36
# baseline (speedup 1.0000x reference)
"""Causal self-attention with RoPE, tensor-parallel over 8 NeuronCores.

Sharding: 8 cores = 2 (batch) x 4 (head-groups of 4 heads).
Each core computes q/k/v projections for its 4 heads, RoPE, causal
softmax(QK^T)V, and a partial output projection (its rows of Wo).
Host sums the 4 partials per batch and adds bo.

v2 layout/schedule (all activations and weights bf16 on device, psum f32):
  xT      [D, S]          x[b] transposed (host, bf16)
  q,k     [128(hd), h, S] "T-layout", head dim on partitions
  v       [128(s), S/128, h*128]
  scores  [128(k), 512(q)] transposed scores; exp on Act engine
  denom   4x 1-row transposed-ones matmuls per score tile into [128,4]
          psum, DVE-accumulated, reciprocal, DMA-transposed to [1,512],
          Pool-broadcast to [128,512] (mode "tiny"); or the classic
          [1,512] ones-matmul accumulation (mode "ones").
  sched   single A phase (q,k,v per s-block, x loaded once, wv resident)
          then attention blocks with the previous block's Wo projection
          matmuls interleaved to fill the Act-latency bubbles.
"""

import math
import os
import sys

sys.path.insert(0, "/opt/trn_rl_repo")

import numpy as np

import concourse.bass as bass
import concourse.tile as tile
from concourse import bacc, mybir
from concourse.bass import ds, ts

B, S, D = 2, 2048, 2048
H, HD = 16, 128
ROPE_BASE = 10000.0
N_CORES = 8
N_GROUPS = 4          # head groups (tensor-parallel axis)
H_LOC = H // N_GROUPS  # heads per core
LP_MODE = os.environ.get("KBENCH_LP", "quad")   # "quad" | "ones"

QB = 512   # query block (free dim of score tiles)
KB = 128   # key block (partition dim of score tiles)


def build_core_program(Sn, Dm, h_loc, kb_plan, n_masks, lp_mode):
    """One core's program (SPMD-shared). kb_plan[qq] = [(kb, mask_idx|None)]."""
    W = h_loc * HD           # local width of Wq/Wk/Wv (columns), Wo (rows)
    KK = Dm // 128           # contraction subtiles for projections
    NSB = Sn // QB           # 512-wide s blocks
    NMB = Sn // KB           # 128-wide s blocks
    nm = QB // KB            # 128-chunks per q block
    NN = Dm // QB            # 512-wide output column blocks
    f32 = mybir.dt.float32
    bf = mybir.dt.bfloat16

    nc = bacc.Bacc("TRN2", target_bir_lowering=False, debug=False,
                   enable_asserts=True, num_devices=N_CORES)

    xT = nc.dram_tensor("xT", [Dm, Sn], bf, kind="ExternalInput").ap()
    wq = nc.dram_tensor("wq", [Dm, W], bf, kind="ExternalInput").ap()
    wk = nc.dram_tensor("wk", [Dm, W], bf, kind="ExternalInput").ap()
    wv = nc.dram_tensor("wv", [Dm, W], bf, kind="ExternalInput").ap()
    wo = nc.dram_tensor("wo", [W, Dm], bf, kind="ExternalInput").ap()
    bq = nc.dram_tensor("bq", [HD, h_loc], f32, kind="ExternalInput").ap()
    bk = nc.dram_tensor("bk", [HD, h_loc], f32, kind="ExternalInput").ap()
    bv = nc.dram_tensor("bv", [1, W], f32, kind="ExternalInput").ap()
    cos2 = nc.dram_tensor("cos2", [HD, Sn], bf, kind="ExternalInput").ap()
    sinS = nc.dram_tensor("sinS", [HD, Sn], bf, kind="ExternalInput").ap()
    prot = nc.dram_tensor("prot", [HD, HD], bf, kind="ExternalInput").ap()
    if n_masks:
        pmask = nc.dram_tensor("pmask", [n_masks, KB, QB], bf,
                               kind="ExternalInput").ap()
    out = nc.dram_tensor("out", [Sn, Dm], f32, kind="ExternalOutput").ap()

    scale = 1.0 / math.sqrt(HD)

    with tile.TileContext(nc) as tc:
        with (
            tc.tile_pool(name="const", bufs=1) as cpool,
            tc.tile_pool(name="big", bufs=1) as big,
        ):
            # persistent activations
            qb_sb = big.tile([HD, h_loc, Sn], bf, tag="qb")
            kb_sb = big.tile([HD, h_loc, Sn], bf, tag="kb")
            v_sb = big.tile([KB, NMB, W], bf, tag="v")
            wo_sb = big.tile([HD, h_loc, Dm], bf, tag="wo")

            cos2_sb = cpool.tile([HD, Sn], bf, tag="cos2")
            sinS_sb = cpool.tile([HD, Sn], bf, tag="sinS")
            bq_sb = cpool.tile([HD, h_loc], f32, tag="bq")
            bk_sb = cpool.tile([HD, h_loc], f32, tag="bk")
            bv_sb = cpool.tile([1, W], f32, tag="bv")
            ones_f = cpool.tile([128, 1], f32, tag="ones_f")
            nc.gpsimd.memset(ones_f[:], 1.0)
            ones_b = cpool.tile([128, 1], bf, tag="ones_b")
            nc.vector.tensor_copy(ones_b[:], ones_f[:])
            bvb = cpool.tile([128, W], f32, tag="bvb")
            if n_masks:
                mask_sb = cpool.tile([KB, n_masks, QB], bf, tag="mask")

            with (
                tc.tile_pool(name="wa", bufs=1) as wpool,
                tc.tile_pool(name="xa", bufs=2) as xpool,
                tc.tile_pool(name="swp", bufs=4) as spool,
                tc.tile_pool(name="psa", bufs=4, space="PSUM") as psA,
                tc.tile_pool(name="psv", bufs=2, space="PSUM") as psV,
                tc.tile_pool(name="psr", bufs=2, space="PSUM") as psR,
            ):
                wq_sb = wpool.tile([128, KK, W], bf, tag="wqr")
                wk_sb = wpool.tile([128, KK, W], bf, tag="wkr")
                wv_sb = wpool.tile([128, KK, W], bf, tag="wvr")

                # first weight tiles on SP (HWDGE, lowest latency); q/k
                # streams on Pool; late-needed tensors on Act. Pool/Act are
                # otherwise idle in phase A.
                prot_sb = cpool.tile([HD, HD], bf, tag="prot")
                x0 = xpool.tile([128, KK, QB], bf, tag="x")
                nc.sync.dma_start(x0[:, 0, :], xT[ts(0, 128), ts(0, QB)])
                nc.sync.dma_start(wq_sb[:, 0, :], wq[ts(0, 128), :])
                nc.sync.dma_start(wk_sb[:, 0, :], wk[ts(0, 128), :])
                nc.gpsimd.dma_start(bq_sb[:], bq[:])
                nc.gpsimd.dma_start(bk_sb[:], bk[:])
                nc.scalar.dma_start(bv_sb[:], bv[:])
                for kk in range(1, KK):
                    nc.gpsimd.dma_start(wq_sb[:, kk, :], wq[ts(kk, 128), :])
                    nc.gpsimd.dma_start(wk_sb[:, kk, :], wk[ts(kk, 128), :])
                nc.scalar.dma_start(prot_sb[:], prot[:])
                nc.scalar.dma_start(cos2_sb[:], cos2[:])
                nc.scalar.dma_start(sinS_sb[:], sinS[:])
                nc.gpsimd.partition_broadcast(bvb[:], bv_sb[:])
                for kk in range(KK):
                    nc.scalar.dma_start(wv_sb[:, kk, :], wv[ts(kk, 128), :])
                if n_masks:
                    nc.scalar.dma_start(
                        mask_sb[:], pmask.rearrange("n p q -> p n q"))
                for h in range(h_loc):
                    nc.scalar.dma_start(wo_sb[:, h, :], wo[ds(h * HD, HD), :])

                # x tiles: SP queue; per-kk slices for pipelined starts
                # (kk=0 already issued ahead of the weight tiles above)
                x_tiles = []
                for kk in range(1, KK):
                    nc.sync.dma_start(x0[:, kk, :], xT[ts(kk, 128), ts(0, QB)])
                x_tiles.append(x0)

                def rope_chunk(srct, h, sb):
                    # rotate_half via PE with a +-1 permutation matrix (no
                    # cross-partition DMA): rot = prot^T @ q, then
                    # q = q*cos + rot*sin on DVE.
                    sl = ts(sb, QB)
                    rot_ps = psR.tile([HD, QB], f32, tag="rr",
                                      name="rot_ps")
                    nc.tensor.matmul(rot_ps[:], prot_sb[:],
                                     srct[:, h, sl], start=True, stop=True)
                    sw = spool.tile([HD, QB], bf, tag="sw")
                    nc.vector.tensor_mul(sw[:], rot_ps[:], sinS_sb[:, sl])
                    nc.vector.tensor_mul(srct[:, h, sl], srct[:, h, sl],
                                         cos2_sb[:, sl])
                    nc.vector.tensor_add(srct[:, h, sl], srct[:, h, sl],
                                         sw[:])

                for sb in range(NSB):
                    x_cur = x_tiles[sb]
                    if sb + 1 < NSB:
                        xn = xpool.tile([128, KK, QB], bf, tag="x")
                        for kk in range(KK):
                            nc.sync.dma_start(xn[:, kk, :],
                                              xT[ts(kk, 128), ts(sb + 1, QB)])
                        x_tiles.append(xn)
                    def emit_v_m(m):
                        v_ps = psV.tile([KB, W], f32, tag="pv",
                                        name="v_ps")
                        for kk in range(KK):
                            nc.tensor.matmul(v_ps[:],
                                             x_cur[:, kk, ts(m, KB)],
                                             wv_sb[:, kk, :],
                                             start=(kk == 0),
                                             stop=(kk == KK - 1))
                        nc.vector.scalar_tensor_tensor(
                            v_sb[:, sb * nm + m, :], v_ps[:], 0.0,
                            bvb[:], op0=mybir.AluOpType.add,
                            op1=mybir.AluOpType.add)

                    def emit_v():
                        for m in range(nm):
                            emit_v_m(m)

                    def emit_head(h):
                        # q and k interleaved per kk so PE consumption
                        # rate-matches the streaming x tiles at startup
                        q_t = psA.tile([HD, QB], f32, tag="pa", name="q_t")
                        k_t = psA.tile([HD, QB], f32, tag="pa", name="k_t")
                        for kk in range(KK):
                            nc.tensor.matmul(q_t[:], wq_sb[:, kk, ts(h, HD)],
                                             x_cur[:, kk, :],
                                             start=(kk == 0),
                                             stop=(kk == KK - 1))
                            nc.tensor.matmul(k_t[:], wk_sb[:, kk, ts(h, HD)],
                                             x_cur[:, kk, :],
                                             start=(kk == 0),
                                             stop=(kk == KK - 1))
                        for dst, p_t, bias in ((qb_sb, q_t, bq_sb),
                                               (kb_sb, k_t, bk_sb)):
                            nc.vector.tensor_scalar_add(
                                dst[:, h, ts(sb, QB)], p_t[:],
                                bias[:, h, None])
                            rope_chunk(dst, h, sb)

                    if sb == 0:
                        for h in range(h_loc):
                            emit_head(h)
                        emit_v()
                    else:
                        for h in range(h_loc):
                            emit_head(h)
                            emit_v_m(h)

            # ------------- Phase B + C interleaved per q-block ----------
            with (
                tc.tile_pool(name="pb", bufs=6) as ppool,
                tc.tile_pool(name="nb", bufs=2) as npool,
                tc.tile_pool(name="rb", bufs=2) as rbpool,
                tc.tile_pool(name="ac", bufs=2) as acache,
                tc.tile_pool(name="oc", bufs=5) as opool,
                tc.tile_pool(name="pss", bufs=2, space="PSUM") as psS,
                tc.tile_pool(name="pso", bufs=2, space="PSUM") as psO,
                tc.tile_pool(name="psl", bufs=1, space="PSUM") as psL,
                tc.tile_pool(name="psc", bufs=3, space="PSUM") as psC,
            ):
                wo_queue = []

                def emit_wo_one():
                    act_prev, qq_prev, m, n = wo_queue.pop(0)
                    op = psC.tile([KB, QB], f32, tag="c", name="op")
                    for h in range(h_loc):
                        nc.tensor.matmul(op[:],
                                         act_prev[:, h, ts(m, KB)],
                                         wo_sb[:, h, ts(n, QB)],
                                         start=(h == 0),
                                         stop=(h == h_loc - 1))
                    ot = opool.tile([KB, QB], f32, tag="ot")
                    nc.vector.tensor_copy(ot[:], op[:])
                    nc.sync.dma_start(out[ts(qq_prev * nm + m, KB),
                                          ts(n, QB)], ot[:])

                for qq in range(NSB):
                    plan = kb_plan[qq]
                    act = acache.tile([HD, h_loc, QB], bf, tag="act")
                    ntiles = max(1, len(plan) * h_loc)
                    njobs = len(wo_queue)
                    stride = max(1, ntiles // njobs) if njobs else 0
                    tcnt = 0
                    for h in range(h_loc):
                        outp = psO.tile([HD, QB], f32, tag="o", name="outp")
                        lp_ps = psL.tile([1, QB], f32, tag="l",
                                         name="lp_ps")
                        pending = None
                        last = len(plan) - 1
                        # masked diagonal tiles only need q >= kb*KB: skip
                        # the fully-masked [0, off) columns everywhere.
                        offs = [max(0, kb * KB - qq * QB)
                                for kb, _ in plan]
                        # replay the grouping to count lp matmuls exactly
                        n_lp, _cnt = 0, 0
                        for o in offs:
                            if o == 0:
                                _cnt += 1
                                if _cnt == 4:
                                    n_lp, _cnt = n_lp + 1, 0
                            else:
                                n_lp += (1 if _cnt else 0) + 1
                                _cnt = 0
                        n_lp += 1 if _cnt else 0
                        qgroup = []
                        gstate = [0]

                        def emit_lp(ap, off):
                            g = gstate[0]
                            nc.tensor.matmul(lp_ps[0:1, off:], ones_b[:],
                                             ap, start=(g == 0),
                                             stop=(g == n_lp - 1))
                            gstate[0] = g + 1

                        def flush_quad():
                            if not qgroup:
                                return
                            if len(qgroup) == 1:
                                src = qgroup[0]
                            else:
                                acc = ppool.tile([KB, QB], bf, tag="pacc",
                                                 bufs=2)
                                nc.vector.tensor_add(acc[:], qgroup[0][:],
                                                     qgroup[1][:])
                                for t in qgroup[2:]:
                                    nc.vector.tensor_add(acc[:], acc[:],
                                                         t[:])
                                src = acc
                            emit_lp(src[:], 0)
                            qgroup.clear()

                        def emit_av(pt, i, kb, off):
                            nc.tensor.matmul(outp[:, off:],
                                             v_sb[:, kb, ts(h, HD)],
                                             pt[:, off:], start=(i == 0),
                                             stop=(i == last))
                            if lp_mode == "quad":
                                if off == 0:
                                    qgroup.append(pt)
                                    if len(qgroup) == 4:
                                        flush_quad()
                                else:
                                    # keep full-span first: drain the open
                                    # quad group before any narrow write
                                    flush_quad()
                                    emit_lp(pt[:, off:], off)
                            else:
                                nc.tensor.matmul(lp_ps[0:1, off:],
                                                 ones_b[:], pt[:, off:],
                                                 start=(i == 0),
                                                 stop=(i == last))

                        for i, (kb, mi) in enumerate(plan):
                            off = offs[i]
                            sp = psS.tile([KB, QB], f32, tag="s", name="sp")
                            nc.tensor.matmul(sp[:, off:],
                                             kb_sb[:, h, ts(kb, KB)],
                                             qb_sb[:, h,
                                                   ds(qq * QB + off,
                                                      QB - off)],
                                             start=True, stop=True)
                            pt = ppool.tile([KB, QB], bf, tag="p")
                            nc.scalar.activation(
                                pt[:, off:], sp[:, off:],
                                mybir.ActivationFunctionType.Exp,
                                bias=0.0, scale=scale)
                            if mi is not None:
                                nc.vector.tensor_mul(pt[:, off:],
                                                     pt[:, off:],
                                                     mask_sb[:, mi, off:])
                            if pending is not None:
                                emit_av(*pending)
                            pending = (pt, i, kb, off)
                            tcnt += 1
                            if njobs and stride and tcnt % stride == 0 \
                                    and tcnt > 2 and wo_queue:
                                emit_wo_one()
                        emit_av(*pending)
                        if lp_mode == "quad":
                            flush_quad()

                        # normalization for head h
                        recb = rbpool.tile([128, QB], f32, tag="recb")
                        rec = npool.tile([1, QB], f32, tag="rec")
                        nc.vector.reciprocal(rec[:], lp_ps[:])
                        nc.gpsimd.partition_broadcast(recb[:], rec[:])
                        nc.vector.scalar_tensor_tensor(
                            act[:, h, :], outp[:], 1.0, recb[:],
                            op0=mybir.AluOpType.mult,
                            op1=mybir.AluOpType.mult)
                    while wo_queue:
                        emit_wo_one()
                    wo_queue = [(act, qq, m, n)
                                for m in range(nm) for n in range(NN)]
                while wo_queue:
                    emit_wo_one()

    nc.compile()
    return nc


# ---------------------------------------------------------------------------
# Host side
# ---------------------------------------------------------------------------

def _bf16(a):
    import ml_dtypes
    return np.ascontiguousarray(np.asarray(a).astype(ml_dtypes.bfloat16))


def _rope_tables(Sn):
    inv = 1.0 / (ROPE_BASE ** (np.arange(0, HD, 2, dtype=np.float32) / HD))
    ang = np.arange(Sn, dtype=np.float32)[:, None] * inv[None, :]
    cosT = np.cos(ang).T.astype(np.float32)          # [64, S]
    sinT = np.sin(ang).T.astype(np.float32)
    cos2 = np.concatenate([cosT, cosT], 0)           # [128, S]
    sinS = np.concatenate([sinT, sinT], 0)
    # rot = P^T q = [-q2; q1]; P[j, d] = coeff of q[j] in rot[d]
    prot = np.zeros((HD, HD), np.float32)
    half = HD // 2
    for d in range(half):
        prot[d + half, d] = -1.0
        prot[d, d + half] = 1.0
    return (np.ascontiguousarray(cos2), np.ascontiguousarray(sinS),
            np.ascontiguousarray(prot))


def _classify_mask(mask, Sn):
    """-> (kb_plan, mask_tiles). kb_plan[qq] = [(kb, mask_idx|None)]."""
    nq, nk = Sn // QB, Sn // KB
    plan = []
    uniq = {}
    tiles = []
    for qq in range(nq):
        row = []
        for kb in range(nk):
            sub = mask[qq * QB:(qq + 1) * QB, kb * KB:(kb + 1) * KB]
            if sub.max() <= -200.0:
                continue                      # exp() == 0 exactly: skip
            if np.all(sub == 0.0):
                row.append((kb, None))
                continue
            t = np.ascontiguousarray(np.exp(sub.astype(np.float64))
                                     .astype(np.float32).T)  # [KB, QB]
            key = t.tobytes()
            if key not in uniq:
                uniq[key] = len(tiles)
                tiles.append(t)
            row.append((kb, uniq[key]))
        plan.append(row)
    return plan, tiles


_CACHE = {}


def _get_runner(plan_key, Sn, Dm, h_loc, kb_plan, n_masks):
    if plan_key in _CACHE:
        return _CACHE[plan_key]
    nc = build_core_program(Sn, Dm, h_loc, kb_plan, n_masks, LP_MODE)
    runner = _make_pjrt_runner(nc, N_CORES)
    _CACHE[plan_key] = runner
    return runner


def _make_pjrt_runner(nc, n_cores):
    """Persistent jitted SPMD executor (replicates bass2jax.run_bass_via_pjrt
    multi-core path, but reusable across calls for stable timing)."""
    import jax
    from jax.sharding import Mesh, PartitionSpec
    from jax.experimental.shard_map import shard_map
    from concourse.bass2jax import (_bass_exec_p, install_neuronx_cc_hook,
                                    partition_id_tensor)

    install_neuronx_cc_hook()
    pname = nc.partition_id_tensor.name if nc.partition_id_tensor else None
    in_names, out_names, out_avals, zero_outs = [], [], [], []
    for alloc in nc.m.functions[0].allocations:
        if not isinstance(alloc, mybir.MemoryLocationSet):
            continue
        name = alloc.memorylocations[0].name
        if alloc.kind == "ExternalInput":
            if name != pname:
                in_names.append(name)
        elif alloc.kind == "ExternalOutput":
            shape = tuple(alloc.tensor_shape)
            dtype = mybir.dt.np(alloc.dtype)
            out_names.append(name)
            out_avals.append(jax.core.ShapedArray(shape, dtype))
            zero_outs.append(np.zeros(shape, dtype))
    n_params = len(in_names)
    all_names = in_names + out_names
    if pname is not None:
        all_names = all_names + [pname]

    def _body(*args):
        operands = list(args)
        if pname is not None:
            operands.append(partition_id_tensor())
        outs = _bass_exec_p.bind(
            *operands, out_avals=tuple(out_avals), in_names=tuple(all_names),
            out_names=tuple(out_names), lowering_input_output_aliases=(),
            sim_require_finite=True, sim_require_nnan=True, nc=nc)
        return tuple(outs)

    devices = jax.devices()[:n_cores]
    mesh = Mesh(np.asarray(devices), ("core",))
    nin = n_params + len(out_names)
    jfn = jax.jit(shard_map(_body, mesh=mesh,
                            in_specs=(PartitionSpec("core"),) * nin,
                            out_specs=(PartitionSpec("core"),) * len(out_names),
                            check_rep=False),
                  keep_unused=True)

    def run(in_maps):
        concat = [np.concatenate([np.asarray(m[nm]) for m in in_maps], axis=0)
                  for nm in in_names]
        zeros = [np.zeros((n_cores * z.shape[0], *z.shape[1:]), z.dtype)
                 for z in zero_outs]
        outs = jfn(*concat, *zeros)
        return [{nm: np.asarray(outs[i]).reshape(n_cores, *out_avals[i].shape)[c]
                 for i, nm in enumerate(out_names)} for c in range(n_cores)]

    run.jfn = jfn
    run.in_names = in_names
    run.out_names = out_names
    run.zero_outs = zero_outs
    run.nc = nc
    return run


def _prep_in_maps(x, attn_mask, Wq, bq, Wk, bk, Wv, bv, Wo, mask_tiles):
    cos2, sinS, prot = _rope_tables(S)
    Wg = H_LOC * HD
    pm = (np.stack(mask_tiles, 0) if mask_tiles else None)
    cos2b, sinSb, protb = _bf16(cos2), _bf16(sinS), _bf16(prot)
    pmb = _bf16(pm) if pm is not None else None
    in_maps = []
    for c in range(N_CORES):
        b, g = divmod(c, N_GROUPS)
        cs = slice(g * Wg, (g + 1) * Wg)
        m = {
            "xT": _bf16(np.asarray(x)[b].T),
            "wq": _bf16(np.asarray(Wq)[:, cs]),
            "wk": _bf16(np.asarray(Wk)[:, cs]),
            "wv": _bf16(np.asarray(Wv)[:, cs]),
            "wo": _bf16(np.asarray(Wo)[cs, :]),
            "bq": np.ascontiguousarray(
                np.asarray(bq, np.float32)[cs].reshape(H_LOC, HD).T),
            "bk": np.ascontiguousarray(
                np.asarray(bk, np.float32)[cs].reshape(H_LOC, HD).T),
            "bv": np.ascontiguousarray(
                np.asarray(bv, np.float32)[cs][None, :]),
            "cos2": cos2b,
            "sinS": sinSb,
            "prot": protb,
        }
        if pmb is not None:
            m["pmask"] = pmb
        in_maps.append(m)
    return in_maps


def kernel(x, attn_mask, Wq, bq, Wk, bk, Wv, bv, Wo, bo):
    x = np.asarray(x, dtype=np.float32)
    mask = np.asarray(attn_mask, dtype=np.float32).reshape(S, S)
    kb_plan, mask_tiles = _classify_mask(mask, S)
    plan_key = (tuple(tuple(r) for r in kb_plan), len(mask_tiles), LP_MODE)
    runner = _get_runner(plan_key, S, D, H_LOC, kb_plan, len(mask_tiles))
    in_maps = _prep_in_maps(x, mask, np.asarray(Wq), np.asarray(bq),
                            np.asarray(Wk), np.asarray(bk), np.asarray(Wv),
                            np.asarray(bv), np.asarray(Wo), mask_tiles)
    results = runner(in_maps)
    out = np.empty((B, S, D), np.float32)
    for b in range(B):
        acc = results[b * N_GROUPS]["out"].astype(np.float32).copy()
        for g in range(1, N_GROUPS):
            acc += results[b * N_GROUPS + g]["out"]
        out[b] = acc + np.asarray(bo, np.float32)[None, :]
    return out
